# revision 3
# baseline (speedup 1.0000x reference)
"""CRF partition-function kernel for Trainium2 (8 NeuronCores).

Probe/rank-1 splice algorithm:
  logZ = lse(alpha_{T-1}) with alpha_t = D_t E^T alpha_{t-1},
  E = exp(trans - c0), D_t = diag(exp(emit_t - m_t)) (host-centered).
  A product of >=~15 of these positive transfer matrices is numerically
  rank-1 (Perron collapse), so each chunk product P_c (64 factors) is
  fully described by two probe vectors:
      v_c = P_c @ 1   (forward vector scan)
      r_c = P_c^T @ 1 (backward vector scan)
  with P_c ~= v_c r_c^T / (1^T v_c), and
      logZ = log(r_1^T alpha_host) + sum_c log(r_{c+1}^T v_c / 1^T v_c)
             + sum over device factors (m_f + c0) + host-chunk part.
  That replaces the T*NT^3 matrix scan with 2*T*NT^2 of batched
  matvecs.  T=8192 -> 128 chunks of L=64; chunk 0 (63 factors, exact
  BOS start) runs on the host in f64; the other 127 forward and 127
  backward chains run as 8 cores x 32 lockstep chains x 64 rounds.
  Every round per chain: out = W^T v (4 accumulating 128x128 matmuls
  over j-tiles/k-halves), then v' = out * demi-column (VectorE, bf16).
  Backward chains use transposed weights and a one-column emission
  shift so all 8 cores run the IDENTICAL program on different inputs.

Dtypes: weights fp8e4 (rescaled so max ~200), state bf16, emissions
bf16, PSUM f32.  Validated error vs f64 reference: ~-5 nats on logZ
~53616 (tolerance is rel 2e-2 ~= 1070 nats).
"""

import numpy as np
import ml_dtypes

import concourse.bass as bass
import concourse.bacc as bacc
import concourse.mybir as mybir
import concourse.tile as tile
from concourse.bass_utils import run_bass_kernel_spmd

BF16 = ml_dtypes.bfloat16
FP8 = ml_dtypes.float8_e4m3

NT = 256
T_FULL = 8192
N_CORES = 8
P = 128
L = 64            # rounds (chunk length)
C = T_FULL // L   # 128 chunks (chunk 0 on host)
NCH = 32          # chain slots per core
RB = 8            # demi round-blocks (L/RB rounds each)
W_DT = "fp8"      # "fp8" | "bf16"

_CACHE = {}


def build_nc(nonce=""):
    """Per-core program: 32 chains x 64 rounds of (4 matmuls + 2 scales)."""
    f32 = mybir.dt.float32
    bf16 = mybir.dt.bfloat16
    wdt = mybir.dt.float8e4 if W_DT == "fp8" else bf16
    RL = L // RB  # rounds per demi block

    nc = bacc.Bacc(None, target_bir_lowering=False)
    # weights: w[kh][jt] = lhsT block [128,128]
    w = nc.declare_dram_parameter("w" + nonce, [2 * P, 2 * P], wdt, isOutput=False)
    # emissions, round-major: demi[h][p, i*NCH + ch], split in RB blocks
    demi = nc.declare_dram_parameter("demi", [NT, L * NCH], bf16, isOutput=False)
    q0 = nc.declare_dram_parameter("q0", [NT, NCH], bf16, isOutput=False)
    qout = nc.declare_dram_parameter("qout", [NT, NCH], bf16, isOutput=True)

    with tile.TileContext(nc) as tc:
        with (
            tc.tile_pool(name="const", bufs=1) as cp,
            tc.tile_pool(name="state", bufs=1) as sp,
            tc.tile_pool(name="ps0", bufs=2, space=bass.MemorySpace.PSUM) as pp0,
            tc.tile_pool(name="ps1", bufs=2, space=bass.MemorySpace.PSUM) as pp1,
        ):
            W = [[cp.tile([P, P], wdt, tag=f"w{kh}{jt}", name=f"w{kh}{jt}")
                  for jt in range(2)] for kh in range(2)]
            for kh in range(2):
                for jt in range(2):
                    nc.sync.dma_start(W[kh][jt][:],
                                      w[kh * P:(kh + 1) * P, jt * P:(jt + 1) * P])

            D = [[cp.tile([P, RL * NCH], bf16, tag=f"d{h}{b}", name=f"d{h}{b}")
                  for b in range(RB)] for h in range(2)]
            for h in range(2):
                for b in range(RB):
                    nc.sync.dma_start(
                        D[h][b][:],
                        demi[h * P:(h + 1) * P, b * RL * NCH:(b + 1) * RL * NCH])

            V = [[sp.tile([P, NCH], bf16, tag=f"v{ph}{h}", name=f"v{ph}{h}")
                  for h in range(2)] for ph in range(2)]
            nc.sync.dma_start(V[1][0][:], q0[0:P, :])
            nc.sync.dma_start(V[1][1][:], q0[P:NT, :])

            for i in range(L):
                vp = V[(i + 1) % 2]
                vn = V[i % 2]
                b, ri = divmod(i, RL)
                ps = []
                for jt in range(2):
                    pool = pp0 if jt == 0 else pp1
                    t = pool.tile([P, NCH], f32, tag=f"ps{jt}", name=f"ps{jt}")
                    ps.append(t)
                    nc.tensor.matmul(t[:], W[0][jt][:], vp[0][:],
                                     start=True, stop=False)
                    nc.tensor.matmul(t[:], W[1][jt][:], vp[1][:],
                                     start=False, stop=True)
                for jt in range(2):
                    dcol = D[jt][b][:, ri * NCH:(ri + 1) * NCH]
                    nc.vector.tensor_mul(vn[jt][:], ps[jt][:], dcol)

            ph = (L - 1) % 2
            nc.sync.dma_start(qout[0:P, :], V[ph][0][:])
            nc.sync.dma_start(qout[P:NT, :], V[ph][1][:])

    nc.compile()
    return nc


def _get_nc(nonce=""):
    if nonce not in _CACHE:
        _CACHE[nonce] = build_nc(nonce)
    return _CACHE[nonce]


def host_prep(emit, trans, BOS):
    """f64 host prep: constants, chunk-0 scan, per-core input maps.

    Core roles: cores 0-3 forward chains (chunks 1..127 -> slot c-1),
    cores 4-7 backward chains (same mapping).  Slot 127 (core 3/7
    chain 31) is a benign dummy.
    """
    emit = emit.astype(np.float64)
    trans = trans.astype(np.float64)
    BOS = BOS.astype(np.float64)

    c0 = float(np.log(np.exp(trans).sum(0).mean()))
    E = np.exp(trans - c0)
    wk = 0.0
    if W_DT == "fp8":
        wk = float(np.floor(np.log2(200.0 / E.max())))
    Es = E * (2.0 ** wk)

    m_f = np.log(np.exp(emit).mean(axis=1))            # [T]
    D = np.exp(emit - m_f[:, None]) * (2.0 ** -wk)     # [T, NT]

    # ---- host chunk 0: factors 1..L-1 exact, log domain ----
    a = BOS + emit[0]
    for f in range(1, L):
        z = trans + a[:, None]
        mm = z.max(axis=0)
        a = emit[f] + np.log(np.exp(z - mm).sum(axis=0)) + mm
    a0m = float(a.max())
    v_host = np.exp(a - a0m)                           # alpha after f=L-1

    wdt = FP8 if W_DT == "fp8" else BF16
    w_fwd = Es.astype(wdt)       # lhsT = E
    w_bwd = Es.T.astype(wdt)     # lhsT = E^T

    # demi layouts, round-major: [NT, L*NCH], col = i*NCH + ch
    demi_f = np.ones((4, NT, L, NCH), dtype=np.float64)
    demi_b = np.ones((4, NT, L, NCH), dtype=np.float64)
    q0_f = np.ones((NT, 4, NCH), dtype=np.float64)
    q0_b = np.ones((NT, 4, NCH), dtype=np.float64)
    for c in range(1, C):
        slot = c - 1
        core, ch = divmod(slot, NCH)
        f0 = c * L
        # forward: round i scale = D[f0+i]
        demi_f[core, :, :, ch] = D[f0:f0 + L].T
        # backward: init = D[f0+L-1]; round i<L-1 scale = D[f0+L-2-i]
        q0_b[:, core, ch] = D[f0 + L - 1]
        demi_b[core, :, :L - 1, ch] = D[f0:f0 + L - 1][::-1].T
        # round L-1 scale stays ones
    in_maps = []
    for core in range(N_CORES):
        if core < 4:
            wmat, dem, q0v = w_fwd, demi_f[core], q0_f[:, core]
        else:
            wmat, dem, q0v = w_bwd, demi_b[core - 4], q0_b[:, core - 4]
        in_maps.append({
            "w": np.ascontiguousarray(wmat),
            "demi": np.ascontiguousarray(
                dem.reshape(NT, L * NCH)).astype(BF16),
            "q0": np.ascontiguousarray(q0v).astype(BF16),
        })
    return in_maps, dict(c0=c0, m_f=m_f, a0m=a0m, v_host=v_host)


def host_combine(results, aux):
    """f64 splice of probe vectors into logZ."""
    c0, m_f, a0m, v_host = aux["c0"], aux["m_f"], aux["a0m"], aux["v_host"]
    v = [None] * C
    r = [None] * C
    for c in range(1, C):
        slot = c - 1
        core, ch = divmod(slot, NCH)
        v[c] = results[core]["qout"][:, ch].astype(np.float64)
        r[c] = results[4 + core]["qout"][:, ch].astype(np.float64)
    acc = a0m
    v_prev = v_host
    for c in range(1, C):
        f0 = c * L
        acc += float(np.log(r[c] @ v_prev))
        acc += float((m_f[f0:f0 + L] + c0).sum())
        v_prev = v[c] / float(v[c].sum())
    acc += float(np.log(v_prev.sum()))
    return acc


def gold_score(emit, y, trans, BOS, EOS):
    e = emit.astype(np.float64)
    t = trans.astype(np.float64)
    yy = np.asarray(y).astype(np.int64)
    T = e.shape[0]
    s = float(BOS[yy[0]])
    s += t[yy[:-1], yy[1:]].sum()
    s += e[np.arange(T - 1), yy[:-1]].sum()
    s += float(EOS[yy[-1]]) + e[T - 1, yy[-1]]
    return s


def kernel(emit, y, trans, BOS, EOS):
    emit = np.asarray(emit)
    trans = np.asarray(trans)
    BOS = np.asarray(BOS)
    EOS = np.asarray(EOS)
    nc = _get_nc()
    in_maps, aux = host_prep(emit, trans, BOS)
    results = run_bass_kernel_spmd(nc, in_maps, list(range(N_CORES))).results
    logZ = host_combine(results, aux)
    gold = gold_score(emit, y, trans, BOS, EOS)
    return np.array(np.float32(logZ - gold))


def prof_setup(inputs, nonce="p1"):
    """Hook for profile_hw: fresh-NEFF nc + per-core in_maps."""
    emit = np.asarray(inputs["emit"])
    trans = np.asarray(inputs["trans"])
    BOS = np.asarray(inputs["BOS"])
    nc = _get_nc(nonce)
    in_maps, _ = host_prep(emit, trans, BOS)
    if nonce:
        for m in in_maps:
            m["w" + nonce] = m.pop("w")
    return nc, in_maps


# revision 5
# speedup vs baseline: 2.3800x; 2.3800x over previous
"""CRF partition-function kernel for Trainium2 (8 NeuronCores).

Probe/rank-1 splice algorithm:
  logZ = lse(alpha_{T-1}) with alpha_t = D_t E^T alpha_{t-1},
  E = exp(trans - c0), D_t = diag(exp(emit_t - m_t)) (host-centered).
  A product of >=~10 of these positive transfer matrices is numerically
  rank-1 (Perron collapse), so each chunk product P_c (L=8 factors) is
  fully described by two probe vectors:
      v_c = P_c @ 1   (forward vector scan)
      r_c = P_c^T @ 1 (backward vector scan)
  with P_c ~= v_c r_c^T / (1^T v_c), and
      logZ = log(r_1^T alpha_host) + sum_c log(r_{c+1}^T v_c / 1^T v_c)
             + sum over device factors (m_f + c0) + host-chunk part.
  This replaces the T*NT^3 matrix scan with 2*T*NT^2 of batched
  matvecs.  T=8192 -> 1024 chunks of L=8; chunk 0 (7 factors, exact
  BOS start) runs on the host in f64; the other 1023 forward and 1023
  backward chains run as 8 cores x 256 lockstep chains x 8 rounds
  (cores 0-3 forward, 4-7 backward; identical program, different
  inputs -- the backward recurrence z <- E(d*z) is re-shaped to
  MM-then-scale by a one-column emission shift).

Round (per chain group g of 128 chains): one PSUM tile [128, 256]
accumulates 4 matmuls (2 j-tiles x 2 k-halves, weight-grouped so each
LDWEIGHTS serves both groups), then one VectorE tensor_mul applies the
per-(state,chain) emission column and writes the bf16 state.

Dtypes: weights fp8e4 (rescaled so max ~200; compensation folded into
emissions), state bf16, emissions bf16, PSUM f32.  Validated vs f64
reference: ~-4.9 nats on logZ ~53616 (tolerance 2e-2 rel ~= 1070).
"""

import numpy as np
import ml_dtypes

import concourse.bass as bass
import concourse.bacc as bacc
import concourse.mybir as mybir
import concourse.tile as tile
from concourse.bass_utils import run_bass_kernel_spmd

BF16 = ml_dtypes.bfloat16
FP8 = ml_dtypes.float8_e4m3

NT = 256
T_FULL = 8192
N_CORES = 8
P = 128
L = 8             # rounds (chunk length)
C = T_FULL // L   # 1024 chunks (chunk 0 on host)
NCH = 256         # chain slots per core
G = 2             # chain groups (latency hiding)
GCH = NCH // G
RB = 2            # demi round-blocks per group
RL = L // RB
W_DT = "fp8"      # "fp8" | "bf16"

_CACHE = {}


def build_nc(nonce=""):
    f32 = mybir.dt.float32
    bf16 = mybir.dt.bfloat16
    wdt = mybir.dt.float8e4 if W_DT == "fp8" else bf16
    GW = 2 * GCH          # state/psum width per group
    DBW = RL * GW         # demi block width

    nc = bacc.Bacc(None, target_bir_lowering=False)
    # w: col = (kh*2+jt)*128 + j'   (lhsT blocks)
    w = nc.declare_dram_parameter("w" + nonce, [P, 4 * P], wdt, isOutput=False)
    # demi: col = g*(L*GW) + i*GW + h*GCH + ch
    demi = nc.declare_dram_parameter("demi", [P, G * L * GW], bf16, isOutput=False)
    # q0/qout: col = g*GW + h*GCH + ch
    q0 = nc.declare_dram_parameter("q0", [P, G * GW], bf16, isOutput=False)
    qout = nc.declare_dram_parameter("qout", [P, G * GW], bf16, isOutput=True)

    with tile.TileContext(nc) as tc:
        with (
            tc.tile_pool(name="const", bufs=1) as cp,
            tc.tile_pool(name="state", bufs=1) as sp,
            tc.tile_pool(name="psA", bufs=2, space=bass.MemorySpace.PSUM) as ppA,
            tc.tile_pool(name="psB", bufs=2, space=bass.MemorySpace.PSUM) as ppB,
        ):
            Wt = cp.tile([P, 4 * P], wdt, tag="w", name="w")
            nc.sync.dma_start(Wt[:], w[:, :])

            S = [[sp.tile([P, GW], bf16, tag=f"s{g}{ph}", name=f"s{g}{ph}")
                  for ph in range(2)] for g in range(G)]
            for g in range(G):
                nc.sync.dma_start(S[g][1][:], q0[:, g * GW:(g + 1) * GW])

            # demi blocks: issue on scalar/gpsimd queues, b-major so both
            # groups' block 0 land first
            D = [[cp.tile([P, DBW], bf16, tag=f"d{g}{b}", name=f"d{g}{b}")
                  for b in range(RB)] for g in range(G)]
            for b in range(RB):
                for g in range(G):
                    eng = nc.scalar if g == 0 else nc.gpsimd
                    eng.dma_start(
                        D[g][b][:],
                        demi[:, g * L * GW + b * DBW:
                             g * L * GW + (b + 1) * DBW])

            pools = [ppA, ppB]
            for i in range(L):
                b, ri = divmod(i, RL)
                PS = [pools[g].tile([P, GW], f32, tag=f"ps{g}", name=f"ps{g}")
                      for g in range(G)]
                for kh, jt in ((0, 0), (1, 0), (0, 1), (1, 1)):
                    wsl = Wt[:, (kh * 2 + jt) * P:(kh * 2 + jt + 1) * P]
                    for g in range(G):
                        nc.tensor.matmul(
                            PS[g][:, jt * GCH:(jt + 1) * GCH],
                            wsl,
                            S[g][(i + 1) % 2][:, kh * GCH:(kh + 1) * GCH],
                            start=(kh == 0), stop=(kh == 1))
                for g in range(G):
                    nc.vector.tensor_mul(
                        S[g][i % 2][:], PS[g][:],
                        D[g][b][:, ri * GW:(ri + 1) * GW])

            ph = (L - 1) % 2
            for g in range(G):
                nc.sync.dma_start(qout[:, g * GW:(g + 1) * GW], S[g][ph][:])

    nc.compile()
    return nc


def _get_nc(nonce=""):
    if nonce not in _CACHE:
        _CACHE[nonce] = build_nc(nonce)
    return _CACHE[nonce]


def host_prep(emit, trans, BOS):
    """f64 host prep: constants, chunk-0 scan, per-core input maps."""
    emit = emit.astype(np.float64)
    trans = trans.astype(np.float64)
    BOS = BOS.astype(np.float64)

    c0 = float(np.log(np.exp(trans).sum(0).mean()))
    E = np.exp(trans - c0)
    wk = 0.0
    if W_DT == "fp8":
        wk = float(np.floor(np.log2(200.0 / E.max())))
    Es = E * (2.0 ** wk)

    m_f = np.log(np.exp(emit).mean(axis=1))            # [T]
    D = np.exp(emit - m_f[:, None]) * (2.0 ** -wk)     # [T, NT]

    # host chunk 0: factors 1..L-1 exact, log domain
    a = BOS + emit[0]
    for f in range(1, L):
        z = trans + a[:, None]
        mm = z.max(axis=0)
        a = emit[f] + np.log(np.exp(z - mm).sum(axis=0)) + mm
    a0m = float(a.max())
    v_host = np.exp(a - a0m)

    wdt = FP8 if W_DT == "fp8" else BF16

    def wlayout(lhsT):
        # [P, 4P], col = (kh*2+jt)*128 + j'
        out = np.empty((P, 4 * P), dtype=np.float64)
        for kh in range(2):
            for jt in range(2):
                out[:, (kh * 2 + jt) * P:(kh * 2 + jt + 1) * P] = \
                    lhsT[kh * P:(kh + 1) * P, jt * P:(jt + 1) * P]
        return out.astype(wdt)

    w_fwd = wlayout(Es)      # lhsT = E
    w_bwd = wlayout(Es.T)    # lhsT = E^T

    idx = np.arange(L)
    in_maps = []
    for core in range(N_CORES):
        fwd = core < 4
        k = core % 4
        # local slot s -> chunk c = k*NCH + s + 1 (clamped; slot 1023 dummy)
        chunks = np.minimum(k * NCH + np.arange(NCH) + 1, C - 1)
        f0 = chunks * L                                 # [NCH]
        if fwd:
            dall = D[(f0[:, None] + idx)]               # [NCH, L, NT]
            q0v = np.ones((NCH, NT))
        else:
            dall = np.ones((NCH, L, NT))
            dall[:, :L - 1, :] = D[(f0[:, None] + (L - 2 - np.arange(L - 1)))]
            q0v = D[f0 + L - 1]                         # [NCH, NT]
        # demi layout [P, g*(L*GW) + i*GW + h*GCH + ch]
        X = dall.reshape(G, GCH, L, 2, P)               # [g,ch,i,h,p]
        dem = np.ascontiguousarray(
            X.transpose(4, 0, 2, 3, 1).reshape(P, G * L * 2 * GCH))
        Q = q0v.reshape(G, GCH, 2, P)                   # [g,ch,h,p]
        q0m = np.ascontiguousarray(
            Q.transpose(3, 0, 2, 1).reshape(P, G * 2 * GCH))
        in_maps.append({
            "w": np.ascontiguousarray(w_fwd if fwd else w_bwd),
            "demi": dem.astype(BF16),
            "q0": q0m.astype(BF16),
        })
    return in_maps, dict(c0=c0, m_f=m_f, a0m=a0m, v_host=v_host)


def host_combine(results, aux):
    """f64 splice of probe vectors into logZ."""
    c0, m_f, a0m, v_host = aux["c0"], aux["m_f"], aux["a0m"], aux["v_host"]
    # gather vectors: qout [P, g*GW + h*GCH + ch] -> v[c][j=h*128+p]
    vs = np.empty((2, C, NT))
    for d in range(2):
        for core in range(4):
            q = results[d * 4 + core]["qout"].astype(np.float64)
            Q = q.reshape(P, G, 2, GCH).transpose(1, 3, 2, 0)  # [g,ch,h,p]
            Qr = Q.reshape(NCH, NT)
            c_start = core * NCH + 1
            n = min(NCH, C - c_start)
            vs[d, c_start:c_start + n] = Qr[:n]
    acc = a0m
    v_prev = v_host
    mc = (m_f.reshape(C, L) + c0).sum(axis=1)           # per-chunk constants
    for c in range(1, C):
        acc += float(np.log(vs[1, c] @ v_prev)) + float(mc[c])
        v_prev = vs[0, c] / float(vs[0, c].sum())
    acc += float(np.log(v_prev.sum()))
    return acc


def gold_score(emit, y, trans, BOS, EOS):
    e = emit.astype(np.float64)
    t = trans.astype(np.float64)
    yy = np.asarray(y).astype(np.int64)
    T = e.shape[0]
    s = float(BOS[yy[0]])
    s += t[yy[:-1], yy[1:]].sum()
    s += e[np.arange(T - 1), yy[:-1]].sum()
    s += float(EOS[yy[-1]]) + e[T - 1, yy[-1]]
    return s


def kernel(emit, y, trans, BOS, EOS):
    emit = np.asarray(emit)
    trans = np.asarray(trans)
    BOS = np.asarray(BOS)
    EOS = np.asarray(EOS)
    nc = _get_nc()
    in_maps, aux = host_prep(emit, trans, BOS)
    results = run_bass_kernel_spmd(nc, in_maps, list(range(N_CORES))).results
    logZ = host_combine(results, aux)
    gold = gold_score(emit, y, trans, BOS, EOS)
    return np.array(np.float32(logZ - gold))


def prof_setup(inputs, nonce="p1"):
    """Hook for profile_hw: fresh-NEFF nc + per-core in_maps."""
    nc = _get_nc(nonce)
    in_maps, _ = host_prep(np.asarray(inputs["emit"]),
                           np.asarray(inputs["trans"]),
                           np.asarray(inputs["BOS"]))
    if nonce:
        for m in in_maps:
            m["w" + nonce] = m.pop("w")
    return nc, in_maps


# revision 8
# speedup vs baseline: 2.4771x; 1.0408x over previous
"""CRF partition-function kernel for Trainium2 (8 NeuronCores).

Probe/rank-1 splice algorithm:
  logZ = lse(alpha_{T-1}) with alpha_t = D_t E^T alpha_{t-1},
  E = exp(trans - c0), D_t = diag(exp(emit_t - m_t)) (host-centered).
  A product of >=~10 of these positive transfer matrices is numerically
  rank-1 (Perron collapse), so each chunk product P_c (L=8 factors) is
  fully described by two probe vectors:
      v_c = P_c @ 1   (forward vector scan)
      r_c = P_c^T @ 1 (backward vector scan)
  with P_c ~= v_c r_c^T / (1^T v_c), and
      logZ = log(r_1^T alpha_host) + sum_c log(r_{c+1}^T v_c / 1^T v_c)
             + sum over device factors (m_f + c0) + host-chunk part.
  This replaces the T*NT^3 matrix scan with 2*T*NT^2 of batched
  matvecs.  T=8192 -> 1024 chunks of L=8; chunk 0 (7 factors, exact
  BOS start) runs on the host in f64; the other 1023 forward and 1023
  backward chains run as 8 cores x 256 lockstep chains x 8 rounds
  (cores 0-3 forward, 4-7 backward; identical program, different
  inputs -- the backward recurrence z <- E(d*z) is re-shaped to
  MM-then-scale by a one-column emission shift).

Round (per chain group g of 128 chains): one PSUM tile [128, 256]
accumulates 4 matmuls (2 j-tiles x 2 k-halves, weight-grouped so each
LDWEIGHTS serves both groups), then one VectorE tensor_mul applies the
per-(state,chain) emission column and writes the bf16 state.

Dtypes: weights fp8e4 (rescaled so max ~200; compensation folded into
emissions), state bf16, emissions bf16, PSUM f32.  Validated vs f64
reference: ~-4.9 nats on logZ ~53616 (tolerance 2e-2 rel ~= 1070).
"""

import numpy as np
import ml_dtypes

import concourse.bass as bass
import concourse.bacc as bacc
import concourse.mybir as mybir
import concourse.tile as tile
from concourse.bass_utils import run_bass_kernel_spmd

BF16 = ml_dtypes.bfloat16
FP8 = ml_dtypes.float8_e4m3

NT = 256
T_FULL = 8192
N_CORES = 8
P = 128
L = 4             # rounds (chunk length)
C = T_FULL // L   # 2048 chunks (chunk 0 on host)
NCH = 512         # chain slots per core
G = 2             # chain groups (latency hiding)
GCH = NCH // G
RB = 2            # demi round-blocks per group
RL = L // RB
W_DT = "fp8"      # "fp8" | "bf16"

_CACHE = {}


def build_nc(nonce=""):
    f32 = mybir.dt.float32
    bf16 = mybir.dt.bfloat16
    wdt = mybir.dt.float8e4 if W_DT == "fp8" else bf16
    GW = 2 * GCH          # state/psum width per group
    DBW = RL * GW         # demi block width

    nc = bacc.Bacc(None, target_bir_lowering=False)
    # w: col = (kh*2+jt)*128 + j'   (lhsT blocks)
    w = nc.declare_dram_parameter("w" + nonce, [P, 4 * P], wdt, isOutput=False)
    # demi: col = g*(L*GW) + i*GW + h*GCH + ch
    demi = nc.declare_dram_parameter("demi", [P, G * L * GW], bf16, isOutput=False)
    # q0/qout: col = g*GW + h*GCH + ch
    q0 = nc.declare_dram_parameter("q0", [P, G * GW], bf16, isOutput=False)
    qout = nc.declare_dram_parameter("qout", [P, G * GW], bf16, isOutput=True)

    with tile.TileContext(nc) as tc:
        with (
            tc.tile_pool(name="const", bufs=1) as cp,
            tc.tile_pool(name="state", bufs=1) as sp,
            tc.tile_pool(name="psA", bufs=2, space=bass.MemorySpace.PSUM) as ppA,
            tc.tile_pool(name="psB", bufs=2, space=bass.MemorySpace.PSUM) as ppB,
        ):
            # PE warmup burst: ~4us of dummy matmuls (no data deps) so the
            # HAM clock gate opens before the real rounds start
            wu = sp.tile([P, 5 * P], bf16, tag="wu", name="wu")
            nc.vector.memset(wu[:], 1.0)
            wups = ppA.tile([P, 4 * P], f32, tag="wups", name="wups")
            for _ in range(9):
                nc.tensor.matmul(wups[:], wu[:, 0:P], wu[:, P:5 * P],
                                 start=True, stop=True)

            Wt = cp.tile([P, 4 * P], wdt, tag="w", name="w")
            nc.sync.dma_start(Wt[:], w[:, :])

            S = [[sp.tile([P, GW], bf16, tag=f"s{g}{ph}", name=f"s{g}{ph}")
                  for ph in range(2)] for g in range(G)]
            q0t = cp.tile([P, G * GW], bf16, tag="q0t", name="q0t")
            nc.sync.dma_start(q0t[:], q0[:, :])

            # demi blocks: issue on scalar/gpsimd queues, b-major so both
            # groups' block 0 land first
            D = [[cp.tile([P, DBW], bf16, tag=f"d{g}{b}", name=f"d{g}{b}")
                  for b in range(RB)] for g in range(G)]
            for b in range(RB):
                for g in range(G):
                    eng = nc.scalar if g == 0 else nc.gpsimd
                    eng.dma_start(
                        D[g][b][:],
                        demi[:, g * L * GW + b * DBW:
                             g * L * GW + (b + 1) * DBW])

            pools = [ppA, ppB]
            for i in range(L):
                b, ri = divmod(i, RL)
                PS = [pools[g].tile([P, GW], f32, tag=f"ps{g}", name=f"ps{g}")
                      for g in range(G)]
                for kh, jt in ((0, 0), (1, 0), (0, 1), (1, 1)):
                    wsl = Wt[:, (kh * 2 + jt) * P:(kh * 2 + jt + 1) * P]
                    for g in range(G):
                        if i == 0:
                            rhs = q0t[:, g * GW + kh * GCH:
                                      g * GW + (kh + 1) * GCH]
                        else:
                            rhs = S[g][(i + 1) % 2][:, kh * GCH:(kh + 1) * GCH]
                        nc.tensor.matmul(
                            PS[g][:, jt * GCH:(jt + 1) * GCH],
                            wsl, rhs,
                            start=(kh == 0), stop=(kh == 1))
                for g in range(G):
                    nc.vector.tensor_mul(
                        S[g][i % 2][:], PS[g][:],
                        D[g][b][:, ri * GW:(ri + 1) * GW])

            ph = (L - 1) % 2
            for g in range(G):
                nc.sync.dma_start(qout[:, g * GW:(g + 1) * GW], S[g][ph][:])

    nc.compile()
    return nc


def _get_nc(nonce=""):
    if nonce not in _CACHE:
        _CACHE[nonce] = build_nc(nonce)
    return _CACHE[nonce]


def host_prep(emit, trans, BOS):
    """f64 host prep: constants, chunk-0 scan, per-core input maps."""
    emit = emit.astype(np.float64)
    trans = trans.astype(np.float64)
    BOS = BOS.astype(np.float64)

    c0 = float(np.log(np.exp(trans).sum(0).mean()))
    E = np.exp(trans - c0)
    wk = 0.0
    if W_DT == "fp8":
        wk = float(np.floor(np.log2(200.0 / E.max())))
    Es = E * (2.0 ** wk)

    m_f = np.log(np.exp(emit).mean(axis=1))            # [T]
    D = np.exp(emit - m_f[:, None]) * (2.0 ** -wk)     # [T, NT]

    # host chunk 0: factors 1..L-1 exact, log domain
    a = BOS + emit[0]
    for f in range(1, L):
        z = trans + a[:, None]
        mm = z.max(axis=0)
        a = emit[f] + np.log(np.exp(z - mm).sum(axis=0)) + mm
    a0m = float(a.max())
    v_host = np.exp(a - a0m)

    wdt = FP8 if W_DT == "fp8" else BF16

    def wlayout(lhsT):
        # [P, 4P], col = (kh*2+jt)*128 + j'
        out = np.empty((P, 4 * P), dtype=np.float64)
        for kh in range(2):
            for jt in range(2):
                out[:, (kh * 2 + jt) * P:(kh * 2 + jt + 1) * P] = \
                    lhsT[kh * P:(kh + 1) * P, jt * P:(jt + 1) * P]
        return out.astype(wdt)

    w_fwd = wlayout(Es)      # lhsT = E
    w_bwd = wlayout(Es.T)    # lhsT = E^T

    idx = np.arange(L)
    in_maps = []
    for core in range(N_CORES):
        fwd = core < 4
        k = core % 4
        # local slot s -> chunk c = k*NCH + s + 1 (clamped; slot 1023 dummy)
        chunks = np.minimum(k * NCH + np.arange(NCH) + 1, C - 1)
        f0 = chunks * L                                 # [NCH]
        if fwd:
            dall = D[(f0[:, None] + idx)]               # [NCH, L, NT]
            q0v = np.ones((NCH, NT))
        else:
            dall = np.ones((NCH, L, NT))
            dall[:, :L - 1, :] = D[(f0[:, None] + (L - 2 - np.arange(L - 1)))]
            q0v = D[f0 + L - 1]                         # [NCH, NT]
        # demi layout [P, g*(L*GW) + i*GW + h*GCH + ch]
        X = dall.reshape(G, GCH, L, 2, P)               # [g,ch,i,h,p]
        dem = np.ascontiguousarray(
            X.transpose(4, 0, 2, 3, 1).reshape(P, G * L * 2 * GCH))
        Q = q0v.reshape(G, GCH, 2, P)                   # [g,ch,h,p]
        q0m = np.ascontiguousarray(
            Q.transpose(3, 0, 2, 1).reshape(P, G * 2 * GCH))
        in_maps.append({
            "w": np.ascontiguousarray(w_fwd if fwd else w_bwd),
            "demi": dem.astype(BF16),
            "q0": q0m.astype(BF16),
        })
    return in_maps, dict(c0=c0, m_f=m_f, a0m=a0m, v_host=v_host)


def host_combine(results, aux):
    """f64 splice of probe vectors into logZ."""
    c0, m_f, a0m, v_host = aux["c0"], aux["m_f"], aux["a0m"], aux["v_host"]
    # gather vectors: qout [P, g*GW + h*GCH + ch] -> v[c][j=h*128+p]
    vs = np.empty((2, C, NT))
    for d in range(2):
        for core in range(4):
            q = results[d * 4 + core]["qout"].astype(np.float64)
            Q = q.reshape(P, G, 2, GCH).transpose(1, 3, 2, 0)  # [g,ch,h,p]
            Qr = Q.reshape(NCH, NT)
            c_start = core * NCH + 1
            n = min(NCH, C - c_start)
            vs[d, c_start:c_start + n] = Qr[:n]
    acc = a0m
    v_prev = v_host
    mc = (m_f.reshape(C, L) + c0).sum(axis=1)           # per-chunk constants
    for c in range(1, C):
        acc += float(np.log(vs[1, c] @ v_prev)) + float(mc[c])
        v_prev = vs[0, c] / float(vs[0, c].sum())
    acc += float(np.log(v_prev.sum()))
    return acc


def gold_score(emit, y, trans, BOS, EOS):
    e = emit.astype(np.float64)
    t = trans.astype(np.float64)
    yy = np.asarray(y).astype(np.int64)
    T = e.shape[0]
    s = float(BOS[yy[0]])
    s += t[yy[:-1], yy[1:]].sum()
    s += e[np.arange(T - 1), yy[:-1]].sum()
    s += float(EOS[yy[-1]]) + e[T - 1, yy[-1]]
    return s


def kernel(emit, y, trans, BOS, EOS):
    emit = np.asarray(emit)
    trans = np.asarray(trans)
    BOS = np.asarray(BOS)
    EOS = np.asarray(EOS)
    nc = _get_nc()
    in_maps, aux = host_prep(emit, trans, BOS)
    results = run_bass_kernel_spmd(nc, in_maps, list(range(N_CORES))).results
    logZ = host_combine(results, aux)
    gold = gold_score(emit, y, trans, BOS, EOS)
    return np.array(np.float32(logZ - gold))


def prof_setup(inputs, nonce="p1"):
    """Hook for profile_hw: fresh-NEFF nc + per-core in_maps."""
    nc = _get_nc(nonce)
    in_maps, _ = host_prep(np.asarray(inputs["emit"]),
                           np.asarray(inputs["trans"]),
                           np.asarray(inputs["BOS"]))
    if nonce:
        for m in in_maps:
            m["w" + nonce] = m.pop("w")
    return nc, in_maps


# revision 13
# speedup vs baseline: 2.6953x; 1.0881x over previous
"""CRF partition-function kernel for Trainium2 (8 NeuronCores).

Probe/rank-1 splice algorithm:
  logZ = lse(alpha_{T-1}) with alpha_t = D_t E^T alpha_{t-1},
  E = exp(trans - c0), D_t = diag(exp(emit_t - m_t)) (host-centered).
  A product of >=~10 of these positive transfer matrices is numerically
  rank-1 (Perron collapse), so each chunk product P_c (L=8 factors) is
  fully described by two probe vectors:
      v_c = P_c @ 1   (forward vector scan)
      r_c = P_c^T @ 1 (backward vector scan)
  with P_c ~= v_c r_c^T / (1^T v_c), and
      logZ = log(r_1^T alpha_host) + sum_c log(r_{c+1}^T v_c / 1^T v_c)
             + sum over device factors (m_f + c0) + host-chunk part.
  This replaces the T*NT^3 matrix scan with 2*T*NT^2 of batched
  matvecs.  T=8192 -> 1024 chunks of L=8; chunk 0 (7 factors, exact
  BOS start) runs on the host in f64; the other 1023 forward and 1023
  backward chains run as 8 cores x 256 lockstep chains x 8 rounds
  (cores 0-3 forward, 4-7 backward; identical program, different
  inputs -- the backward recurrence z <- E(d*z) is re-shaped to
  MM-then-scale by a one-column emission shift).

Round (per chain group g of 128 chains): one PSUM tile [128, 256]
accumulates 4 matmuls (2 j-tiles x 2 k-halves, weight-grouped so each
LDWEIGHTS serves both groups), then one VectorE tensor_mul applies the
per-(state,chain) emission column and writes the bf16 state.

Dtypes: weights fp8e4 (rescaled so max ~200; compensation folded into
emissions), state bf16, emissions bf16, PSUM f32.  Validated vs f64
reference: ~-4.9 nats on logZ ~53616 (tolerance 2e-2 rel ~= 1070).
"""

import numpy as np
import ml_dtypes

import concourse.bass as bass
import concourse.bacc as bacc
import concourse.mybir as mybir
import concourse.tile as tile
from concourse.bass_utils import run_bass_kernel_spmd

BF16 = ml_dtypes.bfloat16
FP8 = ml_dtypes.float8_e4m3

NT = 256
T_FULL = 8192
N_CORES = 8
P = 128
L = 4             # rounds (chunk length)
C = T_FULL // L   # 2048 chunks (chunk 0 on host)
NCH = 512         # chain slots per core
G = 2             # chain groups (latency hiding)
GCH = NCH // G
RB = 2            # demi round-blocks per group
RL = L // RB
W_DT = "fp8"      # "fp8" | "bf16"

_CACHE = {}


def build_nc(nonce=""):
    f32 = mybir.dt.float32
    bf16 = mybir.dt.bfloat16
    wdt = mybir.dt.float8e4 if W_DT == "fp8" else bf16
    GW = 2 * GCH          # state/psum width per group
    DBW = RL * GW         # demi block width

    nc = bacc.Bacc(None, target_bir_lowering=False)
    # w: col = (kh*2+jt)*128 + j'   (lhsT blocks)
    w = nc.declare_dram_parameter("w" + nonce, [P, 4 * P], wdt, isOutput=False)
    # demi: col = g*(L*GW) + i*GW + h*GCH + ch
    demi = nc.declare_dram_parameter("demi", [P, G * L * GW], bf16, isOutput=False)
    # q0/qout: col = g*GW + h*GCH + ch
    q0 = nc.declare_dram_parameter("q0", [P, G * GW], bf16, isOutput=False)
    qout = nc.declare_dram_parameter("qout", [P, G * GW], bf16, isOutput=True)

    with tile.TileContext(nc) as tc:
        with (
            tc.tile_pool(name="const", bufs=1) as cp,
            tc.tile_pool(name="state", bufs=1) as sp,
            tc.tile_pool(name="psA", bufs=2, space=bass.MemorySpace.PSUM) as ppA,
            tc.tile_pool(name="psB", bufs=2, space=bass.MemorySpace.PSUM) as ppB,
        ):
            # PE warmup burst: ~4us of dummy matmuls (no data deps) so the
            # HAM clock gate opens before the real rounds start
            wu = sp.tile([P, 5 * P], bf16, tag="wu", name="wu")
            nc.vector.memset(wu[:], 1.0)
            wups = ppA.tile([P, 4 * P], f32, tag="wups", name="wups")
            for _ in range(11):
                nc.tensor.matmul(wups[:], wu[:, 0:P], wu[:, P:5 * P],
                                 start=True, stop=True)

            Wt = cp.tile([P, 4 * P], wdt, tag="w", name="w")
            nc.sync.dma_start(Wt[:], w[:, :])

            S = [[sp.tile([P, GW], bf16, tag=f"s{g}{ph}", name=f"s{g}{ph}")
                  for ph in range(2)] for g in range(G)]
            q0t = cp.tile([P, G * GW], bf16, tag="q0t", name="q0t")
            nc.sync.dma_start(q0t[:], q0[:, :])
            OUT = sp.tile([P, G * GW], bf16, tag="out", name="out")

            # demi blocks: issue on scalar/gpsimd queues, b-major so both
            # groups' block 0 land first
            D = [[cp.tile([P, DBW], bf16, tag=f"d{g}{b}", name=f"d{g}{b}")
                  for b in range(RB)] for g in range(G)]
            dq = {(0, 0): nc.scalar, (1, 0): nc.gpsimd,
                  (0, 1): nc.sync, (1, 1): nc.scalar}
            for b in range(RB):
                for g in range(G):
                    dq[(g, b % 2)].dma_start(
                        D[g][b][:],
                        demi[:, g * L * GW + b * DBW:
                             g * L * GW + (b + 1) * DBW])

            pools = [ppA, ppB]
            for i in range(L):
                b, ri = divmod(i, RL)
                PS = [pools[g].tile([P, GW], f32, tag=f"ps{g}", name=f"ps{g}")
                      for g in range(G)]
                for kh, jt in ((0, 0), (1, 0), (0, 1), (1, 1)):
                    wsl = Wt[:, (kh * 2 + jt) * P:(kh * 2 + jt + 1) * P]
                    for g in range(G):
                        if i == 0:
                            rhs = q0t[:, g * GW + kh * GCH:
                                      g * GW + (kh + 1) * GCH]
                        else:
                            rhs = S[g][(i + 1) % 2][:, kh * GCH:(kh + 1) * GCH]
                        nc.tensor.matmul(
                            PS[g][:, jt * GCH:(jt + 1) * GCH],
                            wsl, rhs,
                            start=(kh == 0), stop=(kh == 1))
                for g in range(G):
                    dst = (OUT[:, g * GW:(g + 1) * GW] if i == L - 1
                           else S[g][i % 2][:])
                    nc.vector.tensor_mul(
                        dst, PS[g][:],
                        D[g][b][:, ri * GW:(ri + 1) * GW])

            nc.sync.dma_start(qout[:, :], OUT[:])

    nc.compile()
    return nc


def _get_nc(nonce=""):
    if nonce not in _CACHE:
        _CACHE[nonce] = build_nc(nonce)
    return _CACHE[nonce]


def host_prep(emit, trans, BOS):
    """f64 host prep: constants, chunk-0 scan, per-core input maps."""
    emit = emit.astype(np.float64)
    trans = trans.astype(np.float64)
    BOS = BOS.astype(np.float64)

    c0 = float(np.log(np.exp(trans).sum(0).mean()))
    E = np.exp(trans - c0)
    wk = 0.0
    if W_DT == "fp8":
        wk = float(np.floor(np.log2(200.0 / E.max())))
    Es = E * (2.0 ** wk)

    m_f = np.log(np.exp(emit).mean(axis=1))            # [T]
    D = np.exp(emit - m_f[:, None]) * (2.0 ** -wk)     # [T, NT]

    # host chunk 0: factors 1..L-1 exact, log domain
    a = BOS + emit[0]
    for f in range(1, L):
        z = trans + a[:, None]
        mm = z.max(axis=0)
        a = emit[f] + np.log(np.exp(z - mm).sum(axis=0)) + mm
    a0m = float(a.max())
    v_host = np.exp(a - a0m)

    wdt = FP8 if W_DT == "fp8" else BF16

    def wlayout(lhsT):
        # [P, 4P], col = (kh*2+jt)*128 + j'
        out = np.empty((P, 4 * P), dtype=np.float64)
        for kh in range(2):
            for jt in range(2):
                out[:, (kh * 2 + jt) * P:(kh * 2 + jt + 1) * P] = \
                    lhsT[kh * P:(kh + 1) * P, jt * P:(jt + 1) * P]
        return out.astype(wdt)

    w_fwd = wlayout(Es)      # lhsT = E
    w_bwd = wlayout(Es.T)    # lhsT = E^T

    idx = np.arange(L)
    in_maps = []
    for core in range(N_CORES):
        fwd = core < 4
        k = core % 4
        # local slot s -> chunk c = k*NCH + s + 1 (clamped; slot 1023 dummy)
        chunks = np.minimum(k * NCH + np.arange(NCH) + 1, C - 1)
        f0 = chunks * L                                 # [NCH]
        if fwd:
            dall = D[(f0[:, None] + idx)]               # [NCH, L, NT]
            q0v = np.ones((NCH, NT))
        else:
            dall = np.ones((NCH, L, NT))
            dall[:, :L - 1, :] = D[(f0[:, None] + (L - 2 - np.arange(L - 1)))]
            q0v = D[f0 + L - 1]                         # [NCH, NT]
        # demi layout [P, g*(L*GW) + i*GW + h*GCH + ch]
        X = dall.reshape(G, GCH, L, 2, P)               # [g,ch,i,h,p]
        dem = np.ascontiguousarray(
            X.transpose(4, 0, 2, 3, 1).reshape(P, G * L * 2 * GCH))
        Q = q0v.reshape(G, GCH, 2, P)                   # [g,ch,h,p]
        q0m = np.ascontiguousarray(
            Q.transpose(3, 0, 2, 1).reshape(P, G * 2 * GCH))
        in_maps.append({
            "w": np.ascontiguousarray(w_fwd if fwd else w_bwd),
            "demi": dem.astype(BF16),
            "q0": q0m.astype(BF16),
        })
    return in_maps, dict(c0=c0, m_f=m_f, a0m=a0m, v_host=v_host)


def host_combine(results, aux):
    """f64 splice of probe vectors into logZ."""
    c0, m_f, a0m, v_host = aux["c0"], aux["m_f"], aux["a0m"], aux["v_host"]
    # gather vectors: qout [P, g*GW + h*GCH + ch] -> v[c][j=h*128+p]
    vs = np.empty((2, C, NT))
    for d in range(2):
        for core in range(4):
            q = results[d * 4 + core]["qout"].astype(np.float64)
            Q = q.reshape(P, G, 2, GCH).transpose(1, 3, 2, 0)  # [g,ch,h,p]
            Qr = Q.reshape(NCH, NT)
            c_start = core * NCH + 1
            n = min(NCH, C - c_start)
            vs[d, c_start:c_start + n] = Qr[:n]
    acc = a0m
    v_prev = v_host
    mc = (m_f.reshape(C, L) + c0).sum(axis=1)           # per-chunk constants
    for c in range(1, C):
        acc += float(np.log(vs[1, c] @ v_prev)) + float(mc[c])
        v_prev = vs[0, c] / float(vs[0, c].sum())
    acc += float(np.log(v_prev.sum()))
    return acc


def gold_score(emit, y, trans, BOS, EOS):
    e = emit.astype(np.float64)
    t = trans.astype(np.float64)
    yy = np.asarray(y).astype(np.int64)
    T = e.shape[0]
    s = float(BOS[yy[0]])
    s += t[yy[:-1], yy[1:]].sum()
    s += e[np.arange(T - 1), yy[:-1]].sum()
    s += float(EOS[yy[-1]]) + e[T - 1, yy[-1]]
    return s


def kernel(emit, y, trans, BOS, EOS):
    emit = np.asarray(emit)
    trans = np.asarray(trans)
    BOS = np.asarray(BOS)
    EOS = np.asarray(EOS)
    nc = _get_nc()
    in_maps, aux = host_prep(emit, trans, BOS)
    results = run_bass_kernel_spmd(nc, in_maps, list(range(N_CORES))).results
    logZ = host_combine(results, aux)
    gold = gold_score(emit, y, trans, BOS, EOS)
    return np.array(np.float32(logZ - gold))


def prof_setup(inputs, nonce="p1"):
    """Hook for profile_hw: fresh-NEFF nc + per-core in_maps."""
    nc = _get_nc(nonce)
    in_maps, _ = host_prep(np.asarray(inputs["emit"]),
                           np.asarray(inputs["trans"]),
                           np.asarray(inputs["BOS"]))
    if nonce:
        for m in in_maps:
            m["w" + nonce] = m.pop("w")
    return nc, in_maps


# revision 17
# speedup vs baseline: 2.7651x; 1.0259x over previous
"""CRF partition-function kernel for Trainium2 (8 NeuronCores).

Probe/rank-1 splice algorithm:
  logZ = lse(alpha_{T-1}) with alpha_t = D_t E^T alpha_{t-1},
  E = exp(trans - c0), D_t = diag(exp(emit_t - m_t)) (host-centered).
  A product of >=~10 of these positive transfer matrices is numerically
  rank-1 (Perron collapse), so each chunk product P_c (L=8 factors) is
  fully described by two probe vectors:
      v_c = P_c @ 1   (forward vector scan)
      r_c = P_c^T @ 1 (backward vector scan)
  with P_c ~= v_c r_c^T / (1^T v_c), and
      logZ = log(r_1^T alpha_host) + sum_c log(r_{c+1}^T v_c / 1^T v_c)
             + sum over device factors (m_f + c0) + host-chunk part.
  This replaces the T*NT^3 matrix scan with 2*T*NT^2 of batched
  matvecs.  T=8192 -> 1024 chunks of L=8; chunk 0 (7 factors, exact
  BOS start) runs on the host in f64; the other 1023 forward and 1023
  backward chains run as 8 cores x 256 lockstep chains x 8 rounds
  (cores 0-3 forward, 4-7 backward; identical program, different
  inputs -- the backward recurrence z <- E(d*z) is re-shaped to
  MM-then-scale by a one-column emission shift).

Round (per chain group g of 128 chains): one PSUM tile [128, 256]
accumulates 4 matmuls (2 j-tiles x 2 k-halves, weight-grouped so each
LDWEIGHTS serves both groups), then one VectorE tensor_mul applies the
per-(state,chain) emission column and writes the bf16 state.

Dtypes: weights fp8e4 (rescaled so max ~200; compensation folded into
emissions), state bf16, emissions bf16, PSUM f32.  Validated vs f64
reference: ~-4.9 nats on logZ ~53616 (tolerance 2e-2 rel ~= 1070).
"""

import numpy as np
import ml_dtypes

import concourse.bass as bass
import concourse.bacc as bacc
import concourse.mybir as mybir
import concourse.tile as tile
from concourse.bass_utils import run_bass_kernel_spmd

BF16 = ml_dtypes.bfloat16
FP8 = ml_dtypes.float8_e4m3

NT = 256
T_FULL = 8192
N_CORES = 8
P = 128
L = 4             # rounds (chunk length)
C = T_FULL // L   # 2048 chunks (chunk 0 on host)
NCH = 512         # chain slots per core
G = 2             # chain groups (latency hiding)
GCH = NCH // G
RB = 2            # demi round-blocks per group
RL = L // RB
W_DT = "fp8"      # "fp8" | "bf16"

_CACHE = {}


def build_nc(nonce=""):
    f32 = mybir.dt.float32
    bf16 = mybir.dt.bfloat16
    wdt = mybir.dt.float8e4 if W_DT == "fp8" else bf16
    GW = 2 * GCH          # state/psum width per group
    DBW = RL * GW         # demi block width

    nc = bacc.Bacc(None, target_bir_lowering=False)
    # w: col = (kh*2+jt)*128 + j'   (lhsT blocks)
    w = nc.declare_dram_parameter("w" + nonce, [P, 4 * P], wdt, isOutput=False)
    # demi: per group [init | round 0 | ... | round L-1] blocks of GW cols,
    # each block col = h*GCH + ch.  Init doubles as the round-0 state.
    demi = nc.declare_dram_parameter(
        "demi", [P, G * (L + 1) * GW], bf16, isOutput=False)
    qout = nc.declare_dram_parameter("qout", [P, G * GW], bf16, isOutput=True)

    with tile.TileContext(nc) as tc:
        with (
            tc.tile_pool(name="const", bufs=1) as cp,
            tc.tile_pool(name="state", bufs=1) as sp,
            tc.tile_pool(name="psA", bufs=2, space=bass.MemorySpace.PSUM) as ppA,
            tc.tile_pool(name="psB", bufs=2, space=bass.MemorySpace.PSUM) as ppB,
        ):
            # PE warmup burst: ~4us of dummy matmuls (no data deps) so the
            # HAM clock gate opens before the real rounds start
            wu = sp.tile([P, 5 * P], bf16, tag="wu", name="wu")
            nc.vector.memset(wu[:], 1.0)
            wups = ppA.tile([P, 4 * P], f32, tag="wups", name="wups")
            for _ in range(4):
                nc.tensor.matmul(wups[:], wu[:, 0:P], wu[:, P:5 * P],
                                 start=True, stop=True)

            Wt = cp.tile([P, 4 * P], wdt, tag="w", name="w")
            nc.sync.dma_start(Wt[:], w[:, :])

            S = [[sp.tile([P, GW], bf16, tag=f"s{g}{ph}", name=f"s{g}{ph}")
                  for ph in range(2)] for g in range(G)]
            OUT = sp.tile([P, G * GW], bf16, tag="out", name="out")

            # demi: per group one queue (scalar=g0, gpsimd=g1), issued in
            # need-order: init (gates round 0), then round blocks
            GB = (L + 1) * GW
            Dinit = [cp.tile([P, GW], bf16, tag=f"di{g}", name=f"di{g}")
                     for g in range(G)]
            D = [[cp.tile([P, DBW], bf16, tag=f"d{g}{b}", name=f"d{g}{b}")
                  for b in range(RB)] for g in range(G)]
            for g in range(G):
                eng = nc.scalar if g == 0 else nc.gpsimd
                eng.dma_start(Dinit[g][:], demi[:, g * GB:g * GB + GW])
                for b in range(RB):
                    eng.dma_start(
                        D[g][b][:],
                        demi[:, g * GB + (1 + b * RL) * GW:
                             g * GB + (1 + (b + 1) * RL) * GW])

            pools = [ppA, ppB]
            for i in range(L):
                b, ri = divmod(i, RL)
                PS = [pools[g].tile([P, GW], f32, tag=f"ps{g}", name=f"ps{g}")
                      for g in range(G)]
                for kh, jt in ((0, 0), (1, 0), (0, 1), (1, 1)):
                    wsl = Wt[:, (kh * 2 + jt) * P:(kh * 2 + jt + 1) * P]
                    for g in range(G):
                        if i == 0:
                            rhs = Dinit[g][:, kh * GCH:(kh + 1) * GCH]
                        else:
                            rhs = S[g][(i + 1) % 2][:, kh * GCH:(kh + 1) * GCH]
                        nc.tensor.matmul(
                            PS[g][:, jt * GCH:(jt + 1) * GCH],
                            wsl, rhs,
                            start=(kh == 0), stop=(kh == 1))
                for g in range(G):
                    dst = (OUT[:, g * GW:(g + 1) * GW] if i == L - 1
                           else S[g][i % 2][:])
                    nc.vector.tensor_mul(
                        dst, PS[g][:],
                        D[g][b][:, ri * GW:(ri + 1) * GW])

            nc.sync.dma_start(qout[:, :], OUT[:])

    nc.compile()
    return nc


def _get_nc(nonce=""):
    if nonce not in _CACHE:
        _CACHE[nonce] = build_nc(nonce)
    return _CACHE[nonce]


def host_prep(emit, trans, BOS):
    """f64 host prep: constants, chunk-0 scan, per-core input maps."""
    emit = emit.astype(np.float64)
    trans = trans.astype(np.float64)
    BOS = BOS.astype(np.float64)

    c0 = float(np.log(np.exp(trans).sum(0).mean()))
    E = np.exp(trans - c0)
    wk = 0.0
    if W_DT == "fp8":
        wk = float(np.floor(np.log2(200.0 / E.max())))
    Es = E * (2.0 ** wk)

    m_f = np.log(np.exp(emit).mean(axis=1))            # [T]
    D = np.exp(emit - m_f[:, None]) * (2.0 ** -wk)     # [T, NT]

    # host chunk 0: factors 1..L-1 exact, log domain
    a = BOS + emit[0]
    for f in range(1, L):
        z = trans + a[:, None]
        mm = z.max(axis=0)
        a = emit[f] + np.log(np.exp(z - mm).sum(axis=0)) + mm
    a0m = float(a.max())
    v_host = np.exp(a - a0m)

    wdt = FP8 if W_DT == "fp8" else BF16

    def wlayout(lhsT):
        # [P, 4P], col = (kh*2+jt)*128 + j'
        out = np.empty((P, 4 * P), dtype=np.float64)
        for kh in range(2):
            for jt in range(2):
                out[:, (kh * 2 + jt) * P:(kh * 2 + jt + 1) * P] = \
                    lhsT[kh * P:(kh + 1) * P, jt * P:(jt + 1) * P]
        return out.astype(wdt)

    w_fwd = wlayout(Es)      # lhsT = E
    w_bwd = wlayout(Es.T)    # lhsT = E^T

    idx = np.arange(L)
    in_maps = []
    for core in range(N_CORES):
        fwd = core < 4
        k = core % 4
        # local slot s -> chunk c = k*NCH + s + 1 (clamped; slot 1023 dummy)
        chunks = np.minimum(k * NCH + np.arange(NCH) + 1, C - 1)
        f0 = chunks * L                                 # [NCH]
        # dall[:, 0] = init vector (round-0 state), dall[:, 1+i] = round-i scale
        dall = np.ones((NCH, L + 1, NT))
        if fwd:
            dall[:, 1:] = D[(f0[:, None] + idx)]
        else:
            dall[:, 0] = D[f0 + L - 1]
            dall[:, 1:L] = D[(f0[:, None] + (L - 2 - np.arange(L - 1)))]
        # demi layout [P, g*((L+1)*GW) + blk*GW + h*GCH + ch]
        X = dall.reshape(G, GCH, L + 1, 2, P)           # [g,ch,blk,h,p]
        dem = np.ascontiguousarray(
            X.transpose(4, 0, 2, 3, 1).reshape(P, G * (L + 1) * 2 * GCH))
        in_maps.append({
            "w": np.ascontiguousarray(w_fwd if fwd else w_bwd),
            "demi": dem.astype(BF16),
        })
    return in_maps, dict(c0=c0, m_f=m_f, a0m=a0m, v_host=v_host)


def host_combine(results, aux):
    """f64 splice of probe vectors into logZ."""
    c0, m_f, a0m, v_host = aux["c0"], aux["m_f"], aux["a0m"], aux["v_host"]
    # gather vectors: qout [P, g*GW + h*GCH + ch] -> v[c][j=h*128+p]
    vs = np.empty((2, C, NT))
    for d in range(2):
        for core in range(4):
            q = results[d * 4 + core]["qout"].astype(np.float64)
            Q = q.reshape(P, G, 2, GCH).transpose(1, 3, 2, 0)  # [g,ch,h,p]
            Qr = Q.reshape(NCH, NT)
            c_start = core * NCH + 1
            n = min(NCH, C - c_start)
            vs[d, c_start:c_start + n] = Qr[:n]
    acc = a0m
    v_prev = v_host
    mc = (m_f.reshape(C, L) + c0).sum(axis=1)           # per-chunk constants
    for c in range(1, C):
        acc += float(np.log(vs[1, c] @ v_prev)) + float(mc[c])
        v_prev = vs[0, c] / float(vs[0, c].sum())
    acc += float(np.log(v_prev.sum()))
    return acc


def gold_score(emit, y, trans, BOS, EOS):
    e = emit.astype(np.float64)
    t = trans.astype(np.float64)
    yy = np.asarray(y).astype(np.int64)
    T = e.shape[0]
    s = float(BOS[yy[0]])
    s += t[yy[:-1], yy[1:]].sum()
    s += e[np.arange(T - 1), yy[:-1]].sum()
    s += float(EOS[yy[-1]]) + e[T - 1, yy[-1]]
    return s


def kernel(emit, y, trans, BOS, EOS):
    emit = np.asarray(emit)
    trans = np.asarray(trans)
    BOS = np.asarray(BOS)
    EOS = np.asarray(EOS)
    nc = _get_nc()
    in_maps, aux = host_prep(emit, trans, BOS)
    results = run_bass_kernel_spmd(nc, in_maps, list(range(N_CORES))).results
    logZ = host_combine(results, aux)
    gold = gold_score(emit, y, trans, BOS, EOS)
    return np.array(np.float32(logZ - gold))


def prof_setup(inputs, nonce="p1"):
    """Hook for profile_hw: fresh-NEFF nc + per-core in_maps."""
    nc = _get_nc(nonce)
    in_maps, _ = host_prep(np.asarray(inputs["emit"]),
                           np.asarray(inputs["trans"]),
                           np.asarray(inputs["BOS"]))
    if nonce:
        for m in in_maps:
            m["w" + nonce] = m.pop("w")
    return nc, in_maps
